# revision 10
# baseline (speedup 1.0000x reference)
"""GCN-II style graph convolution on 8 Trainium2 NeuronCores (Bass/Tile).

Computes: out = (1-alpha) * segment_sum(x[adj_col] * adj_val, adj_row, N)
               + alpha * feature

Strategy (fully data-parallel, no collectives):
  - Destination nodes sharded 8 ways; x replicated in every core's DRAM
    (stored f16 for bandwidth + tensor-engine speed; PSUM accumulates
    fp32 and the output is fp32).
  - Host-side index preprocessing: edges partitioned per core by
    (dest block of 128 nodes, source segment of 32768 rows), each group
    padded to whole 128-edge chunks (pad edges gather row 0 / weight 0).
    Blocks are grouped into super-blocks of 8 so each (super-block,
    segment) becomes ONE dma_gather (~2K rows) — SWDGE fixed cost is
    ~1-3us per call, so few big gathers beat many small ones. Gathers
    rotate over 4 SWDGE queues.
  - Per super-block: build ALL scatter matrices in two wide DVE
    tensor_tensor ops (f16 2x mode):
        S = (iota_row == ld_broadcast) * val_broadcast
    then per dest block accumulate matmul(S_chunk^T @ Xg_chunk) in PSUM.
  - alpha*feature enters the same accumulation as matmul(alpha*I, feat).
  - PSUM evacuated via scalar-engine copy, DMA'd to the output shard.
"""

import sys

import numpy as np

_TRN_REPO = "/opt/trn_rl_repo"
if _TRN_REPO not in sys.path:
    sys.path.insert(0, _TRN_REPO)

P = 128  # partitions / chunk size / dest block
NCORES = 8
SEG_SHIFT = 15  # source segment size 32768 (int16-indexable)
SBLK = 8        # dest blocks per super-block (gather granularity)
NQUEUES = 4     # SWDGE queues for gathers
MAXGATHER = 1000000000  # unsplit; single_packet=False handles big calls

F16 = np.float16


def _cdiv(a, b):
    return -(-a // b)


def _preprocess(x, feature, adj_row, adj_col, adj_val, alpha,
                n_cores=NCORES, seg_shift=SEG_SHIFT):
    """Index-only preprocessing: per-core edge partitioning + padding."""
    N, D = x.shape
    E = adj_row.shape[0]
    segsz = 1 << seg_shift
    nseg = _cdiv(N, segsz)
    npc = _cdiv(N, n_cores)          # nodes per core
    nblk = _cdiv(npc, P)             # dest blocks per core
    npad = nblk * P
    nsb = _cdiv(nblk, SBLK)          # super-blocks per core

    core = adj_row // npc
    d = adj_row - core * npc         # dest local to core
    b = d // P                       # dest block
    ld = (d % P).astype(np.float32)  # dest local to block
    s = adj_col >> seg_shift         # source segment

    # edges per (core, block, seg); shared chunk budget = max over cores
    flat = ((core.astype(np.int64) * nblk + b) * nseg + s)
    counts = np.bincount(flat, minlength=n_cores * nblk * nseg)
    counts = counts.reshape(n_cores, nblk, nseg)
    nch = _cdiv(counts.max(axis=0), P)          # [nblk, nseg] chunks

    # slot layout: (super-block, seg, block, chunk)-major so each
    # (super-block, seg) is one contiguous gather
    slot_off = np.zeros((nblk, nseg), dtype=np.int64)
    gathers = []        # per sb: list of (seg, slot_start, n_slots)
    sb_chunk0 = []      # first global chunk of each super-block
    sb_nchunks = []     # chunks per super-block
    off = 0
    for isb in range(nsb):
        blocks = range(isb * SBLK, min((isb + 1) * SBLK, nblk))
        sb_chunk0.append(off // P)
        calls = []
        for ss in range(nseg):
            start = off
            for bb in blocks:
                slot_off[bb, ss] = off
                off += int(nch[bb, ss]) * P
            lo = start
            while lo < off:
                n = min(off - lo, MAXGATHER)
                calls.append((ss, lo, n))
                lo += n
        gathers.append(calls)
        sb_nchunks.append((off // P) - sb_chunk0[-1])
    totslot = off
    ctot = totslot // P

    # scatter each core's edges into its padded slot layout
    idx16 = np.zeros((n_cores, totslot), dtype=np.int16)  # pad: row 0 of seg
    ldv = np.zeros((n_cores, totslot), dtype=np.float32)
    valv = np.zeros((n_cores, totslot), dtype=np.float32)  # pad: weight 0

    order = np.argsort(flat, kind="stable")
    fo = flat[order]
    _, first_idx, grp_cnt = np.unique(fo, return_index=True,
                                      return_counts=True)
    rank = np.arange(E, dtype=np.int64) - np.repeat(first_idx, grp_cnt)
    k_s = (fo // (nblk * nseg)).astype(np.int64)
    bs = fo % (nblk * nseg)
    b_s = (bs // nseg).astype(np.int64)
    s_s = (bs % nseg).astype(np.int64)
    pos = slot_off[b_s, s_s] + rank
    idx16[k_s, pos] = (adj_col[order] - (s_s << seg_shift)).astype(np.int16)
    ldv[k_s, pos] = ld[order]
    valv[k_s, pos] = adj_val[order] * (1.0 - alpha)

    # gather-index tile: idx i of a call -> (partition i%16, col i//16),
    # replicated across the 8 groups of 16 partitions. Call offsets are
    # multiples of 128, so one global wrap equals per-call wraps.
    idx_tile = np.ascontiguousarray(
        np.tile(idx16.reshape(n_cores, totslot // 16, 16).transpose(0, 2, 1),
                (1, 8, 1)))
    # chunk-major metadata: column = chunk, partition = edge within chunk
    ld_tile = np.ascontiguousarray(
        ldv.reshape(n_cores, ctot, P).transpose(0, 2, 1)).astype(F16)
    val_tile = np.ascontiguousarray(
        valv.reshape(n_cores, ctot, P).transpose(0, 2, 1)).astype(F16)

    feat_pad = np.zeros((n_cores, npad, D), dtype=F16)
    for k in range(n_cores):
        lo = k * npc
        hi = min(lo + npc, N)
        feat_pad[k, : hi - lo] = feature[lo:hi].astype(F16)

    cmax = max(sb_nchunks)
    iota_big = np.tile(np.arange(P, dtype=np.float32), (P, cmax))
    iota_big = np.ascontiguousarray(iota_big.reshape(P, cmax * P)).astype(F16)
    alpha_eye = np.ascontiguousarray(
        (alpha * np.eye(P, dtype=np.float32)).astype(F16))

    # per-block chunk lists (global chunk indices, slot order)
    chunks_of_block = [[] for _ in range(nblk)]
    for bb in range(nblk):
        for ss in range(nseg):
            c0 = int(slot_off[bb, ss]) // P
            for j in range(int(nch[bb, ss])):
                chunks_of_block[bb].append(c0 + j)

    meta = dict(N=N, D=D, n_cores=n_cores, npc=npc, nblk=nblk, npad=npad,
                segsz=segsz, nseg=nseg, nsb=nsb, totslot=totslot, ctot=ctot,
                cmax=cmax, gathers=gathers, sb_chunk0=sb_chunk0,
                sb_nchunks=sb_nchunks, chunks_of_block=chunks_of_block)
    x_bf = np.ascontiguousarray(x.astype(F16))
    in_maps = []
    for k in range(n_cores):
        in_maps.append({
            "x": x_bf,
            "feat": feat_pad[k],
            "idx16": idx_tile[k],
            "ld": ld_tile[k],
            "val": val_tile[k],
            "iotab": iota_big,
            "alphaI": alpha_eye,
        })
    return meta, in_maps


def _build(meta):
    """Build + compile the (single, SPMD) Bass program."""
    from contextlib import ExitStack

    import concourse.bacc as bacc
    import concourse.mybir as mybir
    import concourse.tile as tile

    N, D = meta["N"], meta["D"]
    nblk = meta["nblk"]
    nsb = meta["nsb"]
    segsz = meta["segsz"]
    npad = meta["npad"]
    totslot = meta["totslot"]
    ctot = meta["ctot"]
    cmax = meta["cmax"]
    gathers = meta["gathers"]
    sb_chunk0 = meta["sb_chunk0"]
    sb_nchunks = meta["sb_nchunks"]
    chunks_of_block = meta["chunks_of_block"]

    f32 = mybir.dt.float32
    f16 = mybir.dt.float16
    nc = bacc.Bacc("TRN2", target_bir_lowering=False, debug=False,
                   num_swdge_queues=NQUEUES)

    x_t = nc.dram_tensor("x", [N, D], f16, kind="ExternalInput").ap()
    feat_t = nc.dram_tensor("feat", [npad, D], f16,
                            kind="ExternalInput").ap()
    idx_t = nc.dram_tensor("idx16", [P, totslot // 16], mybir.dt.int16,
                           kind="ExternalInput").ap()
    ld_t = nc.dram_tensor("ld", [P, ctot], f16, kind="ExternalInput").ap()
    val_t = nc.dram_tensor("val", [P, ctot], f16, kind="ExternalInput").ap()
    iota_t = nc.dram_tensor("iotab", [P, cmax * P], f16,
                            kind="ExternalInput").ap()
    aI_t = nc.dram_tensor("alphaI", [P, P], f16, kind="ExternalInput").ap()
    out_t = nc.dram_tensor("out", [npad, D], f32, kind="ExternalOutput").ap()

    with tile.TileContext(nc) as tc, ExitStack() as ctx:
        const = ctx.enter_context(tc.tile_pool(name="const", bufs=1))
        iota_s = const.tile([P, cmax, P], f16)
        nc.sync.dma_start(iota_s[:], iota_t.rearrange("p (c e) -> p c e",
                                                      e=P))
        aI_s = const.tile([P, P], f16)
        nc.sync.dma_start(aI_s[:], aI_t[:, :])
        ld_s = const.tile([P, ctot], f16)
        nc.sync.dma_start(ld_s[:], ld_t[:, :])
        val_s = const.tile([P, ctot], f16)
        nc.sync.dma_start(val_s[:], val_t[:, :])
        idx_s = const.tile([P, totslot // 16], mybir.dt.int16)
        nc.sync.dma_start(idx_s[:], idx_t[:, :])

        xg_pool = ctx.enter_context(tc.tile_pool(name="xg", bufs=3))
        sval_pool = ctx.enter_context(tc.tile_pool(name="sv", bufs=3))
        feat_pool = ctx.enter_context(tc.tile_pool(name="ft", bufs=8))
        psum_pool = ctx.enter_context(
            tc.tile_pool(name="ps", bufs=8, space="PSUM"))
        out_pool = ctx.enter_context(tc.tile_pool(name="ob", bufs=8))

        q = 0
        for isb in range(nsb):
            csb = sb_nchunks[isb]
            c0 = sb_chunk0[isb]
            xg = xg_pool.tile([P, max(csb, 1), D], f16, tag="xg")
            for (ss, slot_start, n_slots) in gathers[isb]:
                o = slot_start // P - c0
                seg_lo = ss * segsz
                seg_hi = min(seg_lo + segsz, N)
                nc.gpsimd.dma_gather(
                    xg[:, o:o + n_slots // P, :],
                    x_t[seg_lo:seg_hi, :],
                    idx_s[:, slot_start // 16: (slot_start + n_slots) // 16],
                    n_slots,
                    n_slots,
                    D,
                    queue_num=q,
                    single_packet=False,
                )
                q = (q + 1) % NQUEUES

            # scatter matrices for the whole super-block: two wide DVE ops
            sv = sval_pool.tile([P, max(csb, 1), P], f16, tag="sv")
            if csb > 0:
                ld_bc = ld_s[:, c0:c0 + csb, None].to_broadcast([P, csb, P])
                val_bc = val_s[:, c0:c0 + csb, None].to_broadcast([P, csb, P])
                nc.vector.tensor_tensor(
                    out=sv[:, :csb, :], in0=iota_s[:, :csb, :], in1=ld_bc,
                    op=mybir.AluOpType.is_equal)
                nc.vector.tensor_tensor(
                    out=sv[:, :csb, :], in0=sv[:, :csb, :], in1=val_bc,
                    op=mybir.AluOpType.mult)

            for bb in range(isb * SBLK, min((isb + 1) * SBLK, nblk)):
                chunks = chunks_of_block[bb]
                ft = feat_pool.tile([P, D], f16, tag="ft")
                nc.sync.dma_start(ft[:], feat_t[bb * P:(bb + 1) * P, :])
                ps = psum_pool.tile([P, D], f32, tag="ps")
                nc.tensor.matmul(ps[:], aI_s[:], ft[:], start=True,
                                 stop=(len(chunks) == 0))
                for i, g in enumerate(chunks):
                    lc = g - c0
                    nc.tensor.matmul(ps[:], sv[:, lc, :], xg[:, lc, :],
                                     start=False, stop=(i == len(chunks) - 1))
                ob = out_pool.tile([P, D], f32, tag="ob")
                nc.scalar.copy(ob[:], ps[:])
                nc.sync.dma_start(out_t[bb * P:(bb + 1) * P, :], ob[:])

    nc.compile()
    return nc


_CACHE = {}


def _execute(inputs, trace=False, n_cores=NCORES, seg_shift=SEG_SHIFT):
    from concourse.bass_utils import run_bass_kernel_spmd

    x = np.asarray(inputs["x"], dtype=np.float32)
    feature = np.asarray(inputs["feature"], dtype=np.float32)
    adj_row = np.asarray(inputs["adj_row"], dtype=np.int64)
    adj_col = np.asarray(inputs["adj_col"], dtype=np.int64)
    adj_val = np.asarray(inputs["adj_val"], dtype=np.float32)
    alpha = float(np.asarray(inputs["alpha"]))

    import hashlib
    h = hashlib.sha256()
    for a in (adj_row, adj_col, adj_val):
        h.update(np.ascontiguousarray(a).tobytes())
    h.update(np.float64(alpha).tobytes())
    key = (x.shape, feature.shape, n_cores, seg_shift, h.hexdigest())

    if key in _CACHE:
        nc, meta = _CACHE[key]
        _, in_maps = _preprocess(x, feature, adj_row, adj_col, adj_val,
                                 alpha, n_cores, seg_shift)
    else:
        meta, in_maps = _preprocess(x, feature, adj_row, adj_col, adj_val,
                                    alpha, n_cores, seg_shift)
        nc = _build(meta)
        _CACHE[key] = (nc, meta)

    res = run_bass_kernel_spmd(nc, in_maps, core_ids=list(range(n_cores)),
                               trace=trace)
    npc = meta["npc"]
    N = meta["N"]
    pieces = []
    for k in range(n_cores):
        lo = k * npc
        hi = min(lo + npc, N)
        pieces.append(res.results[k]["out"][: hi - lo])
    out = np.concatenate(pieces, axis=0).astype(np.float32)
    return out, res


def kernel(**inputs):
    out, _ = _execute(inputs, trace=False)
    return out


# revision 18
# speedup vs baseline: 2.4217x; 2.4217x over previous
"""GCN-II style graph convolution on 8 Trainium2 NeuronCores (Bass/Tile).

Computes: out = (1-alpha) * segment_sum(x[adj_col] * adj_val, adj_row, N)
               + alpha * feature

Strategy (fully data-parallel, no collectives):
  - Destination nodes sharded 8 ways; x replicated in every core's DRAM
    (stored f16 for bandwidth + tensor-engine speed; PSUM accumulates
    fp32 and the output is fp32).
  - Host-side index preprocessing: edges partitioned per core by
    (dest block of 128 nodes, source segment of 32768 rows), each group
    padded to whole 128-edge chunks (pad edges gather row 0 / weight 0).
    Blocks are grouped into super-blocks of 8 so each (super-block,
    segment) becomes ONE dma_gather (~2K rows) — SWDGE fixed cost is
    ~1-3us per call, so few big gathers beat many small ones. Gathers
    rotate over 4 SWDGE queues.
  - Per super-block: build ALL scatter matrices in two wide DVE
    tensor_tensor ops (f16 2x mode):
        S = (iota_row == ld_broadcast) * val_broadcast
    then per dest block accumulate matmul(S_chunk^T @ Xg_chunk) in PSUM.
  - alpha*feature enters the same accumulation as matmul(alpha*I, feat).
  - PSUM evacuated via scalar-engine copy, DMA'd to the output shard.
"""

import sys

import numpy as np

_TRN_REPO = "/opt/trn_rl_repo"
if _TRN_REPO not in sys.path:
    sys.path.insert(0, _TRN_REPO)

P = 128  # partitions / chunk size / dest block
NCORES = 8
SEG_LIMIT = 32000  # max rows per source segment (int16-indexable)
SBLK = 8        # dest blocks per super-block (gather granularity)
NQUEUES = 4     # SWDGE queues for gathers
MAXGATHER = 1024  # rows per dma_gather call

F16 = np.float16


def _cdiv(a, b):
    return -(-a // b)


def _preprocess(x, feature, adj_row, adj_col, adj_val, alpha,
                n_cores=NCORES, seg_limit=SEG_LIMIT):
    """Index-only preprocessing: per-core edge partitioning + padding."""
    N, D = x.shape
    E = adj_row.shape[0]
    nseg = _cdiv(N, seg_limit)
    segsz = _cdiv(N, nseg)  # equal segments keep (block, seg) counts even
    npc = _cdiv(N, n_cores)          # nodes per core
    nblk = _cdiv(npc, P)             # dest blocks per core
    npad = nblk * P
    nsb = _cdiv(nblk, SBLK)          # super-blocks per core

    core = adj_row // npc
    d = adj_row - core * npc         # dest local to core
    b = d // P                       # dest block
    ld = (d % P).astype(np.float32)  # dest local to block
    s = adj_col // segsz             # source segment

    # edges per (core, block, seg); shared chunk budget = max over cores
    flat = ((core.astype(np.int64) * nblk + b) * nseg + s)
    counts = np.bincount(flat, minlength=n_cores * nblk * nseg)
    counts = counts.reshape(n_cores, nblk, nseg)
    nch = _cdiv(counts.max(axis=0), P)          # [nblk, nseg] chunks

    # slot layout: (super-block, seg, block, chunk)-major so each
    # (super-block, seg) is one contiguous gather
    slot_off = np.zeros((nblk, nseg), dtype=np.int64)
    gathers = []        # per sb: list of (seg, slot_start, n_slots)
    sb_chunk0 = []      # first global chunk of each super-block
    sb_nchunks = []     # chunks per super-block
    off = 0
    for isb in range(nsb):
        blocks = range(isb * SBLK, min((isb + 1) * SBLK, nblk))
        sb_chunk0.append(off // P)
        calls = []
        for ss in range(nseg):
            start = off
            for bb in blocks:
                slot_off[bb, ss] = off
                off += int(nch[bb, ss]) * P
            lo = start
            while lo < off:
                n = min(off - lo, MAXGATHER)
                calls.append((ss, lo, n))
                lo += n
        gathers.append(calls)
        sb_nchunks.append((off // P) - sb_chunk0[-1])
    totslot = off
    ctot = totslot // P

    # scatter each core's edges into its padded slot layout
    idx16 = np.zeros((n_cores, totslot), dtype=np.int16)  # pad: row 0 of seg
    ldv = np.zeros((n_cores, totslot), dtype=np.float32)
    valv = np.zeros((n_cores, totslot), dtype=np.float32)  # pad: weight 0

    order = np.argsort(flat, kind="stable")
    fo = flat[order]
    _, first_idx, grp_cnt = np.unique(fo, return_index=True,
                                      return_counts=True)
    rank = np.arange(E, dtype=np.int64) - np.repeat(first_idx, grp_cnt)
    k_s = (fo // (nblk * nseg)).astype(np.int64)
    bs = fo % (nblk * nseg)
    b_s = (bs // nseg).astype(np.int64)
    s_s = (bs % nseg).astype(np.int64)
    pos = slot_off[b_s, s_s] + rank
    idx16[k_s, pos] = (adj_col[order] - s_s * segsz).astype(np.int16)
    ldv[k_s, pos] = ld[order]
    valv[k_s, pos] = adj_val[order] * (1.0 - alpha)

    # gather-index tile: idx i of a call -> (partition i%16, col i//16),
    # replicated across the 8 groups of 16 partitions. Call offsets are
    # multiples of 128, so one global wrap equals per-call wraps.
    idx_tile = np.ascontiguousarray(
        np.tile(idx16.reshape(n_cores, totslot // 16, 16).transpose(0, 2, 1),
                (1, 8, 1)))
    # chunk-major metadata: column = chunk, partition = edge within chunk
    ld_tile = np.ascontiguousarray(
        ldv.reshape(n_cores, ctot, P).transpose(0, 2, 1)).astype(F16)
    val_tile = np.ascontiguousarray(
        valv.reshape(n_cores, ctot, P).transpose(0, 2, 1)).astype(F16)

    feat_pad = np.zeros((n_cores, npad, D), dtype=F16)
    for k in range(n_cores):
        lo = k * npc
        hi = min(lo + npc, N)
        feat_pad[k, : hi - lo] = feature[lo:hi].astype(F16)

    cmax = max(sb_nchunks)
    iota_big = np.tile(np.arange(P, dtype=np.float32), (P, cmax))
    iota_big = np.ascontiguousarray(iota_big.reshape(P, cmax * P)).astype(F16)
    alpha_eye = np.ascontiguousarray(
        (alpha * np.eye(P, dtype=np.float32)).astype(F16))

    # per-block chunk lists (global chunk indices, slot order)
    chunks_of_block = [[] for _ in range(nblk)]
    for bb in range(nblk):
        for ss in range(nseg):
            c0 = int(slot_off[bb, ss]) // P
            for j in range(int(nch[bb, ss])):
                chunks_of_block[bb].append(c0 + j)

    meta = dict(N=N, D=D, n_cores=n_cores, npc=npc, nblk=nblk, npad=npad,
                segsz=segsz, nseg=nseg, nsb=nsb, totslot=totslot, ctot=ctot,
                cmax=cmax, gathers=gathers, sb_chunk0=sb_chunk0,
                sb_nchunks=sb_nchunks, chunks_of_block=chunks_of_block)
    x_bf = np.ascontiguousarray(x.astype(F16))
    in_maps = []
    for k in range(n_cores):
        in_maps.append({
            "x": x_bf,
            "feat": feat_pad[k],
            "idx16": idx_tile[k],
            "ld": ld_tile[k],
            "val": val_tile[k],
            "iotab": iota_big,
            "alphaI": alpha_eye,
        })
    return meta, in_maps


def _build(meta):
    """Build + compile the (single, SPMD) Bass program."""
    from contextlib import ExitStack

    import concourse.bacc as bacc
    import concourse.mybir as mybir
    import concourse.tile as tile

    N, D = meta["N"], meta["D"]
    nblk = meta["nblk"]
    nsb = meta["nsb"]
    segsz = meta["segsz"]
    npad = meta["npad"]
    totslot = meta["totslot"]
    ctot = meta["ctot"]
    cmax = meta["cmax"]
    gathers = meta["gathers"]
    sb_chunk0 = meta["sb_chunk0"]
    sb_nchunks = meta["sb_nchunks"]
    chunks_of_block = meta["chunks_of_block"]

    f32 = mybir.dt.float32
    f16 = mybir.dt.float16
    nc = bacc.Bacc("TRN2", target_bir_lowering=False, debug=False,
                   num_swdge_queues=NQUEUES)

    x_t = nc.dram_tensor("x", [N, D], f16, kind="ExternalInput").ap()
    feat_t = nc.dram_tensor("feat", [npad, D], f16,
                            kind="ExternalInput").ap()
    idx_t = nc.dram_tensor("idx16", [P, totslot // 16], mybir.dt.int16,
                           kind="ExternalInput").ap()
    ld_t = nc.dram_tensor("ld", [P, ctot], f16, kind="ExternalInput").ap()
    val_t = nc.dram_tensor("val", [P, ctot], f16, kind="ExternalInput").ap()
    iota_t = nc.dram_tensor("iotab", [P, cmax * P], f16,
                            kind="ExternalInput").ap()
    aI_t = nc.dram_tensor("alphaI", [P, P], f16, kind="ExternalInput").ap()
    out_t = nc.dram_tensor("out", [npad, D], f32, kind="ExternalOutput").ap()

    with tile.TileContext(nc) as tc, ExitStack() as ctx:
        const = ctx.enter_context(tc.tile_pool(name="const", bufs=1))
        iota_s = const.tile([P, cmax, P], f16)
        nc.sync.dma_start(iota_s[:], iota_t.rearrange("p (c e) -> p c e",
                                                      e=P))
        aI_s = const.tile([P, P], f16)
        nc.sync.dma_start(aI_s[:], aI_t[:, :])
        ld_s = const.tile([P, ctot], f16)
        nc.sync.dma_start(ld_s[:], ld_t[:, :])
        val_s = const.tile([P, ctot], f16)
        nc.sync.dma_start(val_s[:], val_t[:, :])
        idx_s = const.tile([P, totslot // 16], mybir.dt.int16)
        nc.sync.dma_start(idx_s[:], idx_t[:, :])

        xg_pool = ctx.enter_context(tc.tile_pool(name="xg", bufs=4))
        sval_pool = ctx.enter_context(tc.tile_pool(name="sv", bufs=4))
        feat_pool = ctx.enter_context(tc.tile_pool(name="ft", bufs=8))
        psum_pool = ctx.enter_context(
            tc.tile_pool(name="ps", bufs=8, space="PSUM"))
        out_pool = ctx.enter_context(tc.tile_pool(name="ob", bufs=8))

        q = 0
        for isb in range(nsb):
            csb = sb_nchunks[isb]
            c0 = sb_chunk0[isb]
            xg = xg_pool.tile([P, max(csb, 1), D], f16, tag="xg")
            for (ss, slot_start, n_slots) in gathers[isb]:
                o = slot_start // P - c0
                seg_lo = ss * segsz
                seg_hi = min(seg_lo + segsz, N)
                nc.gpsimd.dma_gather(
                    xg[:, o:o + n_slots // P, :],
                    x_t[seg_lo:seg_hi, :],
                    idx_s[:, slot_start // 16: (slot_start + n_slots) // 16],
                    n_slots,
                    n_slots,
                    D,
                    queue_num=q,
                    single_packet=False,
                )
                q = (q + 1) % NQUEUES

            # scatter matrices for the whole super-block: two wide DVE ops
            sv = sval_pool.tile([P, max(csb, 1), P], f16, tag="sv")
            if csb > 0:
                ld_bc = ld_s[:, c0:c0 + csb, None].to_broadcast([P, csb, P])
                val_bc = val_s[:, c0:c0 + csb, None].to_broadcast([P, csb, P])
                nc.vector.tensor_tensor(
                    out=sv[:, :csb, :], in0=iota_s[:, :csb, :], in1=ld_bc,
                    op=mybir.AluOpType.is_equal)
                nc.vector.tensor_tensor(
                    out=sv[:, :csb, :], in0=sv[:, :csb, :], in1=val_bc,
                    op=mybir.AluOpType.mult)

            for bb in range(isb * SBLK, min((isb + 1) * SBLK, nblk)):
                chunks = chunks_of_block[bb]
                ft = feat_pool.tile([P, D], f16, tag="ft")
                nc.sync.dma_start(ft[:], feat_t[bb * P:(bb + 1) * P, :])
                ps = psum_pool.tile([P, D], f32, tag="ps")
                nc.tensor.matmul(ps[:], aI_s[:], ft[:], start=True,
                                 stop=(len(chunks) == 0))
                for i, g in enumerate(chunks):
                    lc = g - c0
                    nc.tensor.matmul(ps[:], sv[:, lc, :], xg[:, lc, :],
                                     start=False, stop=(i == len(chunks) - 1))
                ob = out_pool.tile([P, D], f32, tag="ob")
                nc.scalar.copy(ob[:], ps[:])
                nc.sync.dma_start(out_t[bb * P:(bb + 1) * P, :], ob[:])

    nc.compile()
    return nc


_CACHE = {}


def _execute(inputs, trace=False, n_cores=NCORES, seg_limit=SEG_LIMIT):
    from concourse.bass_utils import run_bass_kernel_spmd

    x = np.asarray(inputs["x"], dtype=np.float32)
    feature = np.asarray(inputs["feature"], dtype=np.float32)
    adj_row = np.asarray(inputs["adj_row"], dtype=np.int64)
    adj_col = np.asarray(inputs["adj_col"], dtype=np.int64)
    adj_val = np.asarray(inputs["adj_val"], dtype=np.float32)
    alpha = float(np.asarray(inputs["alpha"]))

    import hashlib
    h = hashlib.sha256()
    for a in (adj_row, adj_col, adj_val):
        h.update(np.ascontiguousarray(a).tobytes())
    h.update(np.float64(alpha).tobytes())
    key = (x.shape, feature.shape, n_cores, seg_limit, h.hexdigest())

    if key in _CACHE:
        nc, meta = _CACHE[key]
        _, in_maps = _preprocess(x, feature, adj_row, adj_col, adj_val,
                                 alpha, n_cores, seg_limit)
    else:
        meta, in_maps = _preprocess(x, feature, adj_row, adj_col, adj_val,
                                    alpha, n_cores, seg_limit)
        nc = _build(meta)
        _CACHE[key] = (nc, meta)

    res = run_bass_kernel_spmd(nc, in_maps, core_ids=list(range(n_cores)),
                               trace=trace)
    npc = meta["npc"]
    N = meta["N"]
    pieces = []
    for k in range(n_cores):
        lo = k * npc
        hi = min(lo + npc, N)
        pieces.append(res.results[k]["out"][: hi - lo])
    out = np.concatenate(pieces, axis=0).astype(np.float32)
    return out, res


def kernel(**inputs):
    out, _ = _execute(inputs, trace=False)
    return out
